# revision 20
# baseline (speedup 1.0000x reference)
"""Trainium2 Bass kernel for CornerBoundingBoxEMDLoss.

For each sample: 8x8 pairwise corner distances, then exact min-cost perfect
matching via meet-in-the-middle (pairs -> quads -> complement pairing), same
math as the reference's 40320-permutation brute force, ~50x less arithmetic.

v3 layout: coord-major [feature, sample], so the distance computation is pure
PE GEMMs with one-hot selection matrices. The pre-matching stage is split
into two 256-sample halves (separate PSUM banks per half) so PE/ACT/DVE
pipeline instead of ping-ponging:

  X [48, 512]      = [pred(i,c); -targ(j,c)] x samples   (bf16, host-packed)
  per half h (256 samples):
    diff = S.T @ X[:,h]      -> psA/psB [96, 256]        (PE)
    sq   = Square(diff)      -> bf16 sbuf                (ACT)
    d2   = R.T @ sq (accum)  -> psC [64, 256]            (PE)
    dist = Sqrt(d2)          -> bf16 sbuf                (ACT)
    L1   = ordering GEMMs    -> psL1 [112, 2*256]        (PE)
    cpy  = Copy(psL1)        -> bf16 sbuf                (ACT)
    m    = TTmin(cpy o0,o1)  -> [112, 256] bf16          (DVE, 2x mode)
  per chunk c (128 samples): L2 GEMMs [128, 840] psum -> DVE min-over-6 ->
    gpsimd A+B add -> DVE min-over-70 -> loss[:, c] bf16
  out: PE-transpose loss [128,4] -> [4,128], ACT copy, one contiguous DMA.

All GEMMs bf16 (1 cyc/row vs fp32's 4 + LOW/HIGH split). Rel err ~5e-3 vs
tolerance 2e-2. Data-parallel across 8 cores, 512 samples each.
"""

import itertools

import numpy as np

import concourse.bacc as bacc
import concourse.mybir as mybir
import concourse.tile as tile

N_CORES = 8
B_TOTAL = 4096
B_CORE = B_TOTAL // N_CORES          # 512
N_CHUNKS = 4
CHUNK = B_CORE // N_CHUNKS           # 128
HALF = B_CORE // 2                   # 256

F32 = mybir.dt.float32
BF16 = mybir.dt.bfloat16

# sr tensor ([96, 320] bf16): S0|S1 in rows 0:48 cols 0:192, R0|R1 in
# rows 0:96 cols 192:320. late tensor ([112, 1064] bf16): l1o0|l1o1|l2.
_SR_W = 320
_L1A = 0           # l1o0: rows 0:64, 112 cols
_L1B = 112         # l1o1: rows 0:64, 112 cols
_L2 = 224          # l2: rows 0:112, 840 cols
_LATE_W = 1064


def _build_constants():
    """One-hot selection matrices, packed into a [128, _W-512] bf16 block."""
    import ml_dtypes

    # S0/S1: diff[(i,j,c), b] = X[(i,c), b] + X[24+(j,c), b]  (targ pre-negated)
    s0 = np.zeros((48, 96), dtype=np.float32)
    s1 = np.zeros((48, 96), dtype=np.float32)
    for i in range(4):
        for j in range(8):
            for c in range(3):
                m = i * 24 + j * 3 + c
                s0[i * 3 + c, m] = 1
                s0[24 + j * 3 + c, m] = 1
                s1[(i + 4) * 3 + c, m] = 1
                s1[24 + j * 3 + c, m] = 1

    # R0/R1: d2[(i,j), b] = sum_c sq[(i,j,c), b]; R1 accumulates the i>=4
    # half into output partitions 32..63 of the same psum bank.
    r0 = np.zeros((96, 64), dtype=np.float32)
    r1 = np.zeros((96, 64), dtype=np.float32)
    for i in range(4):
        for j in range(8):
            for c in range(3):
                r0[i * 24 + j * 3 + c, i * 8 + j] = 1
                r1[i * 24 + j * 3 + c, 32 + i * 8 + j] = 1

    # L1: pred-pair x target-pair sums, both orderings (q = pred pair block)
    pairs = list(itertools.combinations(range(8), 2))            # 28
    pair_idx = {p: i for i, p in enumerate(pairs)}
    pred_pairs = [(0, 1), (2, 3), (4, 5), (6, 7)]
    l1o0 = np.zeros((64, 112), dtype=np.float32)
    l1o1 = np.zeros((64, 112), dtype=np.float32)
    for q, (i0, i1) in enumerate(pred_pairs):
        for p, (a, b) in enumerate(pairs):
            col = q * 28 + p
            l1o0[i0 * 8 + a, col] = 1; l1o0[i1 * 8 + b, col] = 1
            l1o1[i0 * 8 + b, col] = 1; l1o1[i1 * 8 + a, col] = 1

    # L2: quad-split sums. cols 0:420 = A side (pred pairs 0,1 onto 4-subset
    # T), cols 420:840 = B side (pred pairs 2,3 onto complement of T).
    subs4 = list(itertools.combinations(range(8), 4))            # 70
    l2 = np.zeros((112, 840), dtype=np.float32)
    for t, T in enumerate(subs4):
        for s, S in enumerate(itertools.combinations(T, 2)):
            rest = tuple(sorted(set(T) - set(S)))
            l2[0 * 28 + pair_idx[S], t * 6 + s] = 1
            l2[1 * 28 + pair_idx[rest], t * 6 + s] = 1
        TB = tuple(sorted(set(range(8)) - set(T)))
        for s, S in enumerate(itertools.combinations(TB, 2)):
            rest = tuple(sorted(set(TB) - set(S)))
            l2[2 * 28 + pair_idx[S], 420 + t * 6 + s] = 1
            l2[3 * 28 + pair_idx[rest], 420 + t * 6 + s] = 1

    sr = np.zeros((96, _SR_W), dtype=ml_dtypes.bfloat16)
    sr[0:48, 0:96] = s0; sr[0:48, 96:192] = s1
    sr[0:96, 192:256] = r0; sr[0:96, 256:320] = r1
    late = np.zeros((112, _LATE_W), dtype=ml_dtypes.bfloat16)
    late[0:64, _L1A:_L1A + 112] = l1o0
    late[0:64, _L1B:_L1B + 112] = l1o1
    late[0:112, _L2:_L2 + 840] = l2
    return sr, late


def build_nc():
    import os
    use_gps = os.environ.get("V_GPS", "1") == "1"

    nc = bacc.Bacc("TRN2", target_bir_lowering=False, debug=False)

    x_d = nc.dram_tensor("xin", [48, 512], BF16, kind="ExternalInput")
    sr_d = nc.dram_tensor("srin", [96, _SR_W], BF16, kind="ExternalInput")
    late_d = nc.dram_tensor("late", [112, _LATE_W], BF16, kind="ExternalInput")
    id_d = nc.dram_tensor("ident", [128, 128], F32, kind="ExternalInput")
    out_d = nc.dram_tensor("out", [B_CORE], F32, kind="ExternalOutput")

    with tile.TileContext(nc) as tc:
        with (
            tc.tile_pool(name="consts", bufs=1) as cpool,
            tc.tile_pool(name="persist", bufs=1) as ppool,
            tc.tile_pool(name="work", bufs=2) as wpool,
            # 4 one-bank slots: psA/psB per half -> psC per half -> psL1 per
            # half rotate through. 2 two-bank slots: L2 chunks + transpose.
            tc.tile_pool(name="ps_sm", bufs=4, space="PSUM") as pssm,
            tc.tile_pool(name="ps_big", bufs=2, space="PSUM") as psbg,
        ):
            # dummy sqrt first: forces the single act-table load (the sqrt
            # table also covers square+copy) during the input-DMA wait.
            dummy = cpool.tile([128, 2], F32, tag="dummy")
            nc.gpsimd.memset(dummy[:, 0:1], 1.0)
            nc.scalar.activation(dummy[:, 1:2], dummy[:, 0:1],
                                 mybir.ActivationFunctionType.Sqrt)

            xt = cpool.tile([48, 512], BF16, tag="xt")
            srt = cpool.tile([96, _SR_W], BF16, tag="srt")
            late = cpool.tile([112, _LATE_W], BF16, tag="late")
            identt = cpool.tile([128, 128], F32, tag="identt")
            # X and S/R issue from the otherwise-idle DVE/ACT queues, which
            # clear the start barrier ~1.2us before SP does.
            nc.scalar.dma_start(xt[:, :], x_d[:, :])
            nc.gpsimd.dma_start(srt[:, :], sr_d[:, :])
            nc.sync.dma_start(late[:, :], late_d[:, :])
            nc.sync.dma_start(identt[:, :], id_d[:, :])
            cIdf = identt[:, :]

            cX = xt[:, :]
            cS = [srt[0:48, 0:96], srt[0:48, 96:192]]
            cR = [srt[0:96, 192:256], srt[0:96, 256:320]]
            cL1 = [late[0:64, _L1A:_L1A + 112],
                   late[0:64, _L1B:_L1B + 112]]
            cL2 = late[0:112, _L2:_L2 + 840]

            H = [slice(0, HALF), slice(HALF, 2 * HALF)]

            # ---- per-half chain, h0 ops strictly first so half 0 races
            # down to the L2 stage while half 1 fills the engines behind it
            psA = [pssm.tile([96, 512], F32, tag="bank", name=f"psA{h}") for h in range(2)]
            psB = [pssm.tile([96, 512], F32, tag="bank", name=f"psB{h}") for h in range(2)]
            psC = [pssm.tile([64, 512], F32, tag="bank", name=f"psC{h}") for h in range(2)]
            psL = [pssm.tile([112, 512], F32, tag="bank", name=f"psL{h}") for h in range(2)]
            sq0 = [wpool.tile([96, HALF], BF16, tag=f"sq0{h}", name=f"sq0{h}") for h in range(2)]
            sq1 = [wpool.tile([96, HALF], BF16, tag=f"sq1{h}", name=f"sq1{h}") for h in range(2)]
            dist = [wpool.tile([64, HALF], BF16, tag=f"dist{h}", name=f"dist{h}") for h in range(2)]
            m_t = ppool.tile([112, 512], BF16, tag="m")

            for h in range(2):
                nc.tensor.matmul(psA[h][:, 0:HALF], cS[0], cX[:, H[h]],
                                 start=True, stop=True)
                nc.tensor.matmul(psB[h][:, 0:HALF], cS[1], cX[:, H[h]],
                                 start=True, stop=True)
                nc.scalar.activation(sq0[h][:, :], psA[h][:, 0:HALF],
                                     mybir.ActivationFunctionType.Square)
                nc.scalar.activation(sq1[h][:, :], psB[h][:, 0:HALF],
                                     mybir.ActivationFunctionType.Square)
                nc.tensor.matmul(psC[h][:, 0:HALF], cR[0], sq0[h][:, :],
                                 start=True, stop=False)
                nc.tensor.matmul(psC[h][:, 0:HALF], cR[1], sq1[h][:, :],
                                 start=False, stop=True)
                nc.scalar.activation(dist[h][:, :], psC[h][:, 0:HALF],
                                     mybir.ActivationFunctionType.Sqrt)
                nc.tensor.matmul(psL[h][:, 0:HALF], cL1[0], dist[h][:, :],
                                 start=True, stop=True)
                nc.tensor.matmul(psL[h][:, HALF:2 * HALF], cL1[1], dist[h][:, :],
                                 start=True, stop=True)
                v1 = psL[h][:, :].rearrange("p (o b) -> p b o", o=2)
                nc.vector.tensor_reduce(m_t[:, H[h]], v1,
                                        axis=mybir.AxisListType.X,
                                        op=mybir.AluOpType.min)

            # ---- L2 + L3 per chunk of 128 samples ----
            loss = ppool.tile([128, N_CHUNKS], F32, tag="loss")
            sum70 = ppool.tile([128, N_CHUNKS * 70], BF16, tag="sum70")
            for c in range(N_CHUNKS):
                sl = slice(c * CHUNK, (c + 1) * CHUNK)
                ps2 = psbg.tile([128, 1024], F32, tag="big")
                nc.tensor.matmul(ps2[:, 0:420], m_t[:, sl], cL2[:, 0:420],
                                 start=True, stop=True)
                nc.tensor.matmul(ps2[:, 512:932], m_t[:, sl], cL2[:, 420:840],
                                 start=True, stop=True)

                minab = wpool.tile([128, 140], BF16, tag="minab",
                                   name=f"minab{c}")
                v = (ps2[:, :].rearrange("p (h x) -> p h x", h=2)
                     [:, :, 0:420].rearrange("p h (t s) -> p h t s", s=6))
                nc.vector.tensor_reduce(minab[:, :], v,
                                        axis=mybir.AxisListType.X,
                                        op=mybir.AluOpType.min)

                eng = nc.gpsimd if use_gps else nc.vector
                eng.tensor_tensor(sum70[:, c * 70:(c + 1) * 70],
                                  minab[:, 0:70], minab[:, 70:140],
                                  op=mybir.AluOpType.add)

            # single min-over-70 for all four chunks at once
            nc.vector.tensor_reduce(
                loss[:, :], sum70[:, :].rearrange("p (c f) -> p c f", c=N_CHUNKS),
                axis=mybir.AxisListType.X, op=mybir.AluOpType.min)

            # ---- transpose [128, 4] -> [4, 128] on PE, DMA psum -> dram
            psT = psbg.tile([4, 128], F32, tag="big")
            nc.tensor.transpose(psT[:, :], loss[:, :], cIdf)
            lossT = ppool.tile([4, 128], F32, tag="lossT")
            nc.vector.tensor_copy(lossT[:, :], psT[:, :])
            odma = nc.gpsimd if os.environ.get("V_ODMA", "1") == "1" else nc.sync
            odma.dma_start(
                out_d[:].rearrange("(c p) -> c p", p=128), lossT[:, :])

    nc.compile()
    return nc


_NC = None


def _get_nc():
    global _NC
    if _NC is None:
        _NC = build_nc()
    return _NC


def _pack_inputs(pred_corners, target_corners):
    import ml_dtypes

    sr, late = _build_constants()
    pred = np.ascontiguousarray(pred_corners, dtype=np.float32)
    targ = np.ascontiguousarray(target_corners, dtype=np.float32)
    # X rows: 0:24 pred (i*3+c), 24:48 -targ (j*3+c); cols: samples
    xs = np.empty((B_TOTAL, 48), dtype=np.float32)
    xs[:, 0:24] = pred.reshape(B_TOTAL, 24)
    xs[:, 24:48] = -targ.reshape(B_TOTAL, 24)
    xs_bf = xs.astype(ml_dtypes.bfloat16)

    ident = np.eye(128, dtype=np.float32)
    in_maps = []
    for k in range(N_CORES):
        x = np.ascontiguousarray(xs_bf[k * B_CORE:(k + 1) * B_CORE].T)
        in_maps.append({"xin": x, "srin": sr, "late": late, "ident": ident})
    return in_maps


def kernel(pred_corners: np.ndarray, target_corners: np.ndarray) -> np.ndarray:
    from concourse.bass_utils import run_bass_kernel_spmd

    nc = _get_nc()
    in_maps = _pack_inputs(pred_corners, target_corners)
    res = run_bass_kernel_spmd(nc, in_maps, core_ids=list(range(N_CORES)))
    return np.concatenate([res.results[k]["out"] for k in range(N_CORES)])


# revision 21
# speedup vs baseline: 1.0826x; 1.0826x over previous
"""Trainium2 Bass kernel for CornerBoundingBoxEMDLoss.

For each sample: 8x8 pairwise corner distances, then exact min-cost perfect
matching via meet-in-the-middle (pairs -> quads -> complement pairing), same
math as the reference's 40320-permutation brute force, ~50x less arithmetic.

v3 layout: coord-major [feature, sample], so the distance computation is pure
PE GEMMs with one-hot selection matrices. The pre-matching stage is split
into two 256-sample halves (separate PSUM banks per half) so PE/ACT/DVE
pipeline instead of ping-ponging:

  X [48, 512]      = [pred(i,c); -targ(j,c)] x samples   (bf16, host-packed)
  per half h (256 samples):
    diff = S.T @ X[:,h]      -> psA/psB [96, 256]        (PE)
    sq   = Square(diff)      -> bf16 sbuf                (ACT)
    d2   = R.T @ sq (accum)  -> psC [64, 256]            (PE)
    dist = Sqrt(d2)          -> bf16 sbuf                (ACT)
    L1   = ordering GEMMs    -> psL1 [112, 2*256]        (PE)
    cpy  = Copy(psL1)        -> bf16 sbuf                (ACT)
    m    = TTmin(cpy o0,o1)  -> [112, 256] bf16          (DVE, 2x mode)
  per chunk c (128 samples): L2 GEMMs [128, 840] psum -> DVE min-over-6 ->
    gpsimd A+B add -> DVE min-over-70 -> loss[:, c] bf16
  out: PE-transpose loss [128,4] -> [4,128], ACT copy, one contiguous DMA.

All GEMMs bf16 (1 cyc/row vs fp32's 4 + LOW/HIGH split). Rel err ~5e-3 vs
tolerance 2e-2. Data-parallel across 8 cores, 512 samples each.
"""

import itertools

import numpy as np

import concourse.bacc as bacc
import concourse.mybir as mybir
import concourse.tile as tile

N_CORES = 8
B_TOTAL = 4096
B_CORE = B_TOTAL // N_CORES          # 512
N_CHUNKS = 4
CHUNK = B_CORE // N_CHUNKS           # 128
HALF = B_CORE // 2                   # 256

F32 = mybir.dt.float32
BF16 = mybir.dt.bfloat16

# early tensor ([128, 832] bf16): X in rows 0:48 cols 0:512, S0|S1 in rows
# 0:48 cols 512:704, R0|R1 in rows 0:96 cols 704:832.
# late tensor ([112, 1064] bf16): l1o0|l1o1|l2.
_SR_W = 320
_L1A = 0           # l1o0: rows 0:64, 112 cols
_L1B = 112         # l1o1: rows 0:64, 112 cols
_L2 = 224          # l2: rows 0:112, 840 cols
_LATE_W = 1064


def _build_constants():
    """One-hot selection matrices, packed into a [128, _W-512] bf16 block."""
    import ml_dtypes

    # S0/S1: diff[(i,j,c), b] = X[(i,c), b] + X[24+(j,c), b]  (targ pre-negated)
    s0 = np.zeros((48, 96), dtype=np.float32)
    s1 = np.zeros((48, 96), dtype=np.float32)
    for i in range(4):
        for j in range(8):
            for c in range(3):
                m = i * 24 + j * 3 + c
                s0[i * 3 + c, m] = 1
                s0[24 + j * 3 + c, m] = 1
                s1[(i + 4) * 3 + c, m] = 1
                s1[24 + j * 3 + c, m] = 1

    # R0/R1: d2[(i,j), b] = sum_c sq[(i,j,c), b]; R1 accumulates the i>=4
    # half into output partitions 32..63 of the same psum bank.
    r0 = np.zeros((96, 64), dtype=np.float32)
    r1 = np.zeros((96, 64), dtype=np.float32)
    for i in range(4):
        for j in range(8):
            for c in range(3):
                r0[i * 24 + j * 3 + c, i * 8 + j] = 1
                r1[i * 24 + j * 3 + c, 32 + i * 8 + j] = 1

    # L1: pred-pair x target-pair sums, both orderings (q = pred pair block)
    pairs = list(itertools.combinations(range(8), 2))            # 28
    pair_idx = {p: i for i, p in enumerate(pairs)}
    pred_pairs = [(0, 1), (2, 3), (4, 5), (6, 7)]
    l1o0 = np.zeros((64, 112), dtype=np.float32)
    l1o1 = np.zeros((64, 112), dtype=np.float32)
    for q, (i0, i1) in enumerate(pred_pairs):
        for p, (a, b) in enumerate(pairs):
            col = q * 28 + p
            l1o0[i0 * 8 + a, col] = 1; l1o0[i1 * 8 + b, col] = 1
            l1o1[i0 * 8 + b, col] = 1; l1o1[i1 * 8 + a, col] = 1

    # L2: quad-split sums. cols 0:420 = A side (pred pairs 0,1 onto 4-subset
    # T), cols 420:840 = B side (pred pairs 2,3 onto complement of T).
    subs4 = list(itertools.combinations(range(8), 4))            # 70
    l2 = np.zeros((112, 840), dtype=np.float32)
    for t, T in enumerate(subs4):
        for s, S in enumerate(itertools.combinations(T, 2)):
            rest = tuple(sorted(set(T) - set(S)))
            l2[0 * 28 + pair_idx[S], t * 6 + s] = 1
            l2[1 * 28 + pair_idx[rest], t * 6 + s] = 1
        TB = tuple(sorted(set(range(8)) - set(T)))
        for s, S in enumerate(itertools.combinations(TB, 2)):
            rest = tuple(sorted(set(TB) - set(S)))
            l2[2 * 28 + pair_idx[S], 420 + t * 6 + s] = 1
            l2[3 * 28 + pair_idx[rest], 420 + t * 6 + s] = 1

    sr = np.zeros((128, _SR_W), dtype=ml_dtypes.bfloat16)
    sr[0:48, 0:96] = s0; sr[0:48, 96:192] = s1
    sr[0:96, 192:256] = r0; sr[0:96, 256:320] = r1
    late = np.zeros((112, _LATE_W), dtype=ml_dtypes.bfloat16)
    late[0:64, _L1A:_L1A + 112] = l1o0
    late[0:64, _L1B:_L1B + 112] = l1o1
    late[0:112, _L2:_L2 + 840] = l2
    return sr, late


def build_nc():
    import os
    use_gps = os.environ.get("V_GPS", "1") == "1"

    nc = bacc.Bacc("TRN2", target_bir_lowering=False, debug=False)

    early_d = nc.dram_tensor("early", [128, 512 + _SR_W], BF16,
                             kind="ExternalInput")
    late_d = nc.dram_tensor("late", [112, _LATE_W], BF16, kind="ExternalInput")
    id_d = nc.dram_tensor("ident", [128, 128], F32, kind="ExternalInput")
    out_d = nc.dram_tensor("out", [B_CORE], F32, kind="ExternalOutput")

    with tile.TileContext(nc) as tc:
        with (
            tc.tile_pool(name="consts", bufs=1) as cpool,
            tc.tile_pool(name="persist", bufs=1) as ppool,
            tc.tile_pool(name="work", bufs=2) as wpool,
            # 4 one-bank slots: psA/psB per half -> psC per half -> psL1 per
            # half rotate through. 2 two-bank slots: L2 chunks + transpose.
            tc.tile_pool(name="ps_sm", bufs=4, space="PSUM") as pssm,
            tc.tile_pool(name="ps_big", bufs=2, space="PSUM") as psbg,
        ):
            # dummy sqrt first: forces the single act-table load (the sqrt
            # table also covers square+copy) during the input-DMA wait.
            dummy = cpool.tile([128, 2], F32, tag="dummy")
            nc.gpsimd.memset(dummy[:, 0:1], 1.0)
            nc.scalar.activation(dummy[:, 1:2], dummy[:, 0:1],
                                 mybir.ActivationFunctionType.Sqrt)

            early = cpool.tile([128, 512 + _SR_W], BF16, tag="early")
            late = cpool.tile([112, _LATE_W], BF16, tag="late")
            identt = cpool.tile([128, 128], F32, tag="identt")
            nc.sync.dma_start(early[:, :], early_d[:, :])
            nc.sync.dma_start(late[:, :], late_d[:, :])
            nc.sync.dma_start(identt[:, :], id_d[:, :])
            cIdf = identt[:, :]

            cX = early[0:48, 0:512]
            cS = [early[0:48, 512:608], early[0:48, 608:704]]
            cR = [early[0:96, 704:768], early[0:96, 768:832]]
            cL1 = [late[0:64, _L1A:_L1A + 112],
                   late[0:64, _L1B:_L1B + 112]]
            cL2 = late[0:112, _L2:_L2 + 840]

            H = [slice(0, HALF), slice(HALF, 2 * HALF)]

            # ---- per-half chain, h0 ops strictly first so half 0 races
            # down to the L2 stage while half 1 fills the engines behind it
            psA = [pssm.tile([96, 512], F32, tag="bank", name=f"psA{h}") for h in range(2)]
            psB = [pssm.tile([96, 512], F32, tag="bank", name=f"psB{h}") for h in range(2)]
            psC = [pssm.tile([64, 512], F32, tag="bank", name=f"psC{h}") for h in range(2)]
            psL = [pssm.tile([112, 512], F32, tag="bank", name=f"psL{h}") for h in range(2)]
            sq0 = [wpool.tile([96, HALF], BF16, tag=f"sq0{h}", name=f"sq0{h}") for h in range(2)]
            sq1 = [wpool.tile([96, HALF], BF16, tag=f"sq1{h}", name=f"sq1{h}") for h in range(2)]
            dist = [wpool.tile([64, HALF], BF16, tag=f"dist{h}", name=f"dist{h}") for h in range(2)]
            m_t = ppool.tile([112, 512], BF16, tag="m")

            for h in range(2):
                nc.tensor.matmul(psA[h][:, 0:HALF], cS[0], cX[:, H[h]],
                                 start=True, stop=True)
                nc.tensor.matmul(psB[h][:, 0:HALF], cS[1], cX[:, H[h]],
                                 start=True, stop=True)
                nc.scalar.activation(sq0[h][:, :], psA[h][:, 0:HALF],
                                     mybir.ActivationFunctionType.Square)
                nc.scalar.activation(sq1[h][:, :], psB[h][:, 0:HALF],
                                     mybir.ActivationFunctionType.Square)
                nc.tensor.matmul(psC[h][:, 0:HALF], cR[0], sq0[h][:, :],
                                 start=True, stop=False)
                nc.tensor.matmul(psC[h][:, 0:HALF], cR[1], sq1[h][:, :],
                                 start=False, stop=True)
                nc.scalar.activation(dist[h][:, :], psC[h][:, 0:HALF],
                                     mybir.ActivationFunctionType.Sqrt)
                nc.tensor.matmul(psL[h][:, 0:HALF], cL1[0], dist[h][:, :],
                                 start=True, stop=True)
                nc.tensor.matmul(psL[h][:, HALF:2 * HALF], cL1[1], dist[h][:, :],
                                 start=True, stop=True)
                v1 = psL[h][:, :].rearrange("p (o b) -> p b o", o=2)
                nc.vector.tensor_reduce(m_t[:, H[h]], v1,
                                        axis=mybir.AxisListType.X,
                                        op=mybir.AluOpType.min)

            # ---- L2 + L3 per chunk of 128 samples ----
            loss = ppool.tile([128, N_CHUNKS], F32, tag="loss")
            sum70 = ppool.tile([128, N_CHUNKS * 70], BF16, tag="sum70")
            for c in range(N_CHUNKS):
                sl = slice(c * CHUNK, (c + 1) * CHUNK)
                ps2 = psbg.tile([128, 1024], F32, tag="big")
                nc.tensor.matmul(ps2[:, 0:420], m_t[:, sl], cL2[:, 0:420],
                                 start=True, stop=True)
                nc.tensor.matmul(ps2[:, 512:932], m_t[:, sl], cL2[:, 420:840],
                                 start=True, stop=True)

                minab = wpool.tile([128, 140], BF16, tag="minab",
                                   name=f"minab{c}")
                v = (ps2[:, :].rearrange("p (h x) -> p h x", h=2)
                     [:, :, 0:420].rearrange("p h (t s) -> p h t s", s=6))
                nc.vector.tensor_reduce(minab[:, :], v,
                                        axis=mybir.AxisListType.X,
                                        op=mybir.AluOpType.min)

                eng = nc.gpsimd if use_gps else nc.vector
                eng.tensor_tensor(sum70[:, c * 70:(c + 1) * 70],
                                  minab[:, 0:70], minab[:, 70:140],
                                  op=mybir.AluOpType.add)

            # single min-over-70 for all four chunks at once
            nc.vector.tensor_reduce(
                loss[:, :], sum70[:, :].rearrange("p (c f) -> p c f", c=N_CHUNKS),
                axis=mybir.AxisListType.X, op=mybir.AluOpType.min)

            # ---- transpose [128, 4] -> [4, 128] on PE, DMA psum -> dram
            psT = psbg.tile([4, 128], F32, tag="big")
            nc.tensor.transpose(psT[:, :], loss[:, :], cIdf)
            lossT = ppool.tile([4, 128], F32, tag="lossT")
            nc.vector.tensor_copy(lossT[:, :], psT[:, :])
            nc.sync.dma_start(
                out_d[:].rearrange("(c p) -> c p", p=128), lossT[:, :])

    nc.compile()
    return nc


_NC = None


def _get_nc():
    global _NC
    if _NC is None:
        _NC = build_nc()
    return _NC


def _pack_inputs(pred_corners, target_corners):
    import ml_dtypes

    sr, late = _build_constants()
    pred = np.ascontiguousarray(pred_corners, dtype=np.float32)
    targ = np.ascontiguousarray(target_corners, dtype=np.float32)
    # X rows: 0:24 pred (i*3+c), 24:48 -targ (j*3+c); cols: samples
    xs = np.empty((B_TOTAL, 48), dtype=np.float32)
    xs[:, 0:24] = pred.reshape(B_TOTAL, 24)
    xs[:, 24:48] = -targ.reshape(B_TOTAL, 24)
    xs_bf = xs.astype(ml_dtypes.bfloat16)

    ident = np.eye(128, dtype=np.float32)
    in_maps = []
    for k in range(N_CORES):
        early = np.zeros((128, 512 + _SR_W), dtype=ml_dtypes.bfloat16)
        early[0:48, 0:512] = xs_bf[k * B_CORE:(k + 1) * B_CORE].T
        early[:, 512:] = sr
        in_maps.append({"early": early, "late": late, "ident": ident})
    return in_maps


def kernel(pred_corners: np.ndarray, target_corners: np.ndarray) -> np.ndarray:
    from concourse.bass_utils import run_bass_kernel_spmd

    nc = _get_nc()
    in_maps = _pack_inputs(pred_corners, target_corners)
    res = run_bass_kernel_spmd(nc, in_maps, core_ids=list(range(N_CORES)))
    return np.concatenate([res.results[k]["out"] for k in range(N_CORES)])


# revision 22
# speedup vs baseline: 1.1149x; 1.0299x over previous
"""Trainium2 Bass kernel for CornerBoundingBoxEMDLoss.

For each sample: 8x8 pairwise corner distances, then exact min-cost perfect
matching via meet-in-the-middle (pairs -> quads -> complement pairing), same
math as the reference's 40320-permutation brute force, ~50x less arithmetic.

v3 layout: coord-major [feature, sample], so the distance computation is pure
PE GEMMs with one-hot selection matrices. The pre-matching stage is split
into two 256-sample halves (separate PSUM banks per half) so PE/ACT/DVE
pipeline instead of ping-ponging:

  X [48, 512]      = [pred(i,c); -targ(j,c)] x samples   (bf16, host-packed)
  per half h (256 samples):
    diff = S.T @ X[:,h]      -> psA/psB [96, 256]        (PE)
    sq   = Square(diff)      -> bf16 sbuf                (ACT)
    d2   = R.T @ sq (accum)  -> psC [64, 256]            (PE)
    dist = Sqrt(d2)          -> bf16 sbuf                (ACT)
    L1   = ordering GEMMs    -> psL1 [112, 2*256]        (PE)
    cpy  = Copy(psL1)        -> bf16 sbuf                (ACT)
    m    = TTmin(cpy o0,o1)  -> [112, 256] bf16          (DVE, 2x mode)
  per chunk c (128 samples): L2 GEMMs [128, 840] psum -> DVE min-over-6 ->
    gpsimd A+B add -> DVE min-over-70 -> loss[:, c] bf16
  out: PE-transpose loss [128,4] -> [4,128], ACT copy, one contiguous DMA.

All GEMMs bf16 (1 cyc/row vs fp32's 4 + LOW/HIGH split). Rel err ~5e-3 vs
tolerance 2e-2. Data-parallel across 8 cores, 512 samples each.
"""

import itertools

import numpy as np

import concourse.bacc as bacc
import concourse.mybir as mybir
import concourse.tile as tile

N_CORES = 8
B_TOTAL = 4096
B_CORE = B_TOTAL // N_CORES          # 512
N_CHUNKS = 4
CHUNK = B_CORE // N_CHUNKS           # 128
HALF = B_CORE // 2                   # 256

F32 = mybir.dt.float32
BF16 = mybir.dt.bfloat16

# early tensor ([128, 832] bf16): X in rows 0:48 cols 0:512, S0|S1 in rows
# 0:48 cols 512:704, R0|R1 in rows 0:96 cols 704:832.
# late tensor ([112, 1064] bf16): l1o0|l1o1|l2.
_SR_W = 320
_L1A = 0           # l1o0: rows 0:64, 112 cols
_L1B = 112         # l1o1: rows 0:64, 112 cols
_L2 = 224          # l2: rows 0:112, 840 cols
_LATE_W = 1064


def _build_constants():
    """One-hot selection matrices, packed into a [128, _W-512] bf16 block."""
    import ml_dtypes

    # S0/S1: diff[(i,j,c), b] = X[(i,c), b] + X[24+(j,c), b]  (targ pre-negated)
    s0 = np.zeros((48, 96), dtype=np.float32)
    s1 = np.zeros((48, 96), dtype=np.float32)
    for i in range(4):
        for j in range(8):
            for c in range(3):
                m = i * 24 + j * 3 + c
                s0[i * 3 + c, m] = 1
                s0[24 + j * 3 + c, m] = 1
                s1[(i + 4) * 3 + c, m] = 1
                s1[24 + j * 3 + c, m] = 1

    # R0/R1: d2[(i,j), b] = sum_c sq[(i,j,c), b]; R1 accumulates the i>=4
    # half into output partitions 32..63 of the same psum bank.
    r0 = np.zeros((96, 64), dtype=np.float32)
    r1 = np.zeros((96, 64), dtype=np.float32)
    for i in range(4):
        for j in range(8):
            for c in range(3):
                r0[i * 24 + j * 3 + c, i * 8 + j] = 1
                r1[i * 24 + j * 3 + c, 32 + i * 8 + j] = 1

    # L1: pred-pair x target-pair sums, both orderings (q = pred pair block)
    pairs = list(itertools.combinations(range(8), 2))            # 28
    pair_idx = {p: i for i, p in enumerate(pairs)}
    pred_pairs = [(0, 1), (2, 3), (4, 5), (6, 7)]
    l1o0 = np.zeros((64, 112), dtype=np.float32)
    l1o1 = np.zeros((64, 112), dtype=np.float32)
    for q, (i0, i1) in enumerate(pred_pairs):
        for p, (a, b) in enumerate(pairs):
            col = q * 28 + p
            l1o0[i0 * 8 + a, col] = 1; l1o0[i1 * 8 + b, col] = 1
            l1o1[i0 * 8 + b, col] = 1; l1o1[i1 * 8 + a, col] = 1

    # L2: quad-split sums. cols 0:420 = A side (pred pairs 0,1 onto 4-subset
    # T), cols 420:840 = B side (pred pairs 2,3 onto complement of T).
    subs4 = list(itertools.combinations(range(8), 4))            # 70
    l2 = np.zeros((112, 840), dtype=np.float32)
    for t, T in enumerate(subs4):
        for s, S in enumerate(itertools.combinations(T, 2)):
            rest = tuple(sorted(set(T) - set(S)))
            l2[0 * 28 + pair_idx[S], t * 6 + s] = 1
            l2[1 * 28 + pair_idx[rest], t * 6 + s] = 1
        TB = tuple(sorted(set(range(8)) - set(T)))
        for s, S in enumerate(itertools.combinations(TB, 2)):
            rest = tuple(sorted(set(TB) - set(S)))
            l2[2 * 28 + pair_idx[S], 420 + t * 6 + s] = 1
            l2[3 * 28 + pair_idx[rest], 420 + t * 6 + s] = 1

    sr = np.zeros((128, _SR_W), dtype=ml_dtypes.bfloat16)
    sr[0:48, 0:96] = s0; sr[0:48, 96:192] = s1
    sr[0:96, 192:256] = r0; sr[0:96, 256:320] = r1
    late = np.zeros((112, _LATE_W), dtype=ml_dtypes.bfloat16)
    late[0:64, _L1A:_L1A + 112] = l1o0
    late[0:64, _L1B:_L1B + 112] = l1o1
    late[0:112, _L2:_L2 + 840] = l2
    return sr, late


def build_nc():
    import os
    use_gps = os.environ.get("V_GPS", "1") == "1"

    nc = bacc.Bacc("TRN2", target_bir_lowering=False, debug=False)

    early_d = nc.dram_tensor("early", [128, 512 + _SR_W], BF16,
                             kind="ExternalInput")
    late_d = nc.dram_tensor("late", [112, _LATE_W], BF16, kind="ExternalInput")
    id_d = nc.dram_tensor("ident", [128, 128], F32, kind="ExternalInput")
    out_d = nc.dram_tensor("out", [B_CORE], F32, kind="ExternalOutput")

    with tile.TileContext(nc) as tc:
        with (
            tc.tile_pool(name="consts", bufs=1) as cpool,
            tc.tile_pool(name="persist", bufs=1) as ppool,
            tc.tile_pool(name="work", bufs=2) as wpool,
            # 4 one-bank slots: psA/psB per half -> psC per half -> psL1 per
            # half rotate through. 2 two-bank slots: L2 chunks + transpose.
            tc.tile_pool(name="ps_sm", bufs=4, space="PSUM") as pssm,
            tc.tile_pool(name="ps_big", bufs=2, space="PSUM") as psbg,
        ):
            # dummy sqrt first: forces the single act-table load (the sqrt
            # table also covers square+copy) during the input-DMA wait.
            dummy = cpool.tile([128, 2], F32, tag="dummy")
            nc.gpsimd.memset(dummy[:, 0:1], 1.0)
            nc.scalar.activation(dummy[:, 1:2], dummy[:, 0:1],
                                 mybir.ActivationFunctionType.Sqrt)

            early = cpool.tile([128, 512 + _SR_W], BF16, tag="early")
            late = cpool.tile([112, _LATE_W], BF16, tag="late")
            identt = cpool.tile([128, 128], F32, tag="identt")
            nc.sync.dma_start(early[:, :], early_d[:, :])
            nc.sync.dma_start(late[:, :], late_d[:, :])
            nc.sync.dma_start(identt[:, :], id_d[:, :])
            cIdf = identt[:, :]

            cX = early[0:48, 0:512]
            cS = [early[0:48, 512:608], early[0:48, 608:704]]
            cR = [early[0:96, 704:768], early[0:96, 768:832]]
            cL1 = [late[0:64, _L1A:_L1A + 112],
                   late[0:64, _L1B:_L1B + 112]]
            cL2 = late[0:112, _L2:_L2 + 840]

            H = [slice(0, HALF), slice(HALF, 2 * HALF)]

            # ---- per-half chain, h0 ops strictly first so half 0 races
            # down to the L2 stage while half 1 fills the engines behind it
            psA = [pssm.tile([96, 512], F32, tag="bank", name=f"psA{h}") for h in range(2)]
            psB = [pssm.tile([96, 512], F32, tag="bank", name=f"psB{h}") for h in range(2)]
            psC = [pssm.tile([64, 512], F32, tag="bank", name=f"psC{h}") for h in range(2)]
            psL = [pssm.tile([112, 512], F32, tag="bank", name=f"psL{h}") for h in range(2)]
            sq0 = [wpool.tile([96, HALF], BF16, tag=f"sq0{h}", name=f"sq0{h}") for h in range(2)]
            sq1 = [wpool.tile([96, HALF], BF16, tag=f"sq1{h}", name=f"sq1{h}") for h in range(2)]
            dist = [wpool.tile([64, HALF], BF16, tag=f"dist{h}", name=f"dist{h}") for h in range(2)]
            m_t = ppool.tile([112, 512], BF16, tag="m")

            for h in range(2):
                nc.tensor.matmul(psA[h][:, 0:HALF], cS[0], cX[:, H[h]],
                                 start=True, stop=True)
            for h in range(2):
                nc.tensor.matmul(psB[h][:, 0:HALF], cS[1], cX[:, H[h]],
                                 start=True, stop=True)
            for h in range(2):
                nc.scalar.activation(sq0[h][:, :], psA[h][:, 0:HALF],
                                     mybir.ActivationFunctionType.Square)
            for h in range(2):
                nc.scalar.activation(sq1[h][:, :], psB[h][:, 0:HALF],
                                     mybir.ActivationFunctionType.Square)
            for h in range(2):
                nc.tensor.matmul(psC[h][:, 0:HALF], cR[0], sq0[h][:, :],
                                 start=True, stop=False)
            for h in range(2):
                nc.tensor.matmul(psC[h][:, 0:HALF], cR[1], sq1[h][:, :],
                                 start=False, stop=True)
            for h in range(2):
                nc.scalar.activation(dist[h][:, :], psC[h][:, 0:HALF],
                                     mybir.ActivationFunctionType.Sqrt)
            for h in range(2):
                nc.tensor.matmul(psL[h][:, 0:HALF], cL1[0], dist[h][:, :],
                                 start=True, stop=True)
            for h in range(2):
                nc.tensor.matmul(psL[h][:, HALF:2 * HALF], cL1[1], dist[h][:, :],
                                 start=True, stop=True)
            for h in range(2):
                v1 = psL[h][:, :].rearrange("p (o b) -> p b o", o=2)
                nc.vector.tensor_reduce(m_t[:, H[h]], v1,
                                        axis=mybir.AxisListType.X,
                                        op=mybir.AluOpType.min)

            # ---- L2 + L3 per chunk of 128 samples ----
            loss = ppool.tile([128, N_CHUNKS], F32, tag="loss")
            sum70 = ppool.tile([128, N_CHUNKS * 70], BF16, tag="sum70")
            for c in range(N_CHUNKS):
                sl = slice(c * CHUNK, (c + 1) * CHUNK)
                ps2 = psbg.tile([128, 1024], F32, tag="big")
                nc.tensor.matmul(ps2[:, 0:420], m_t[:, sl], cL2[:, 0:420],
                                 start=True, stop=True)
                nc.tensor.matmul(ps2[:, 512:932], m_t[:, sl], cL2[:, 420:840],
                                 start=True, stop=True)

                minab = wpool.tile([128, 140], BF16, tag="minab",
                                   name=f"minab{c}")
                v = (ps2[:, :].rearrange("p (h x) -> p h x", h=2)
                     [:, :, 0:420].rearrange("p h (t s) -> p h t s", s=6))
                nc.vector.tensor_reduce(minab[:, :], v,
                                        axis=mybir.AxisListType.X,
                                        op=mybir.AluOpType.min)

                eng = nc.gpsimd if use_gps else nc.vector
                eng.tensor_tensor(sum70[:, c * 70:(c + 1) * 70],
                                  minab[:, 0:70], minab[:, 70:140],
                                  op=mybir.AluOpType.add)

            # single min-over-70 for all four chunks at once
            nc.vector.tensor_reduce(
                loss[:, :], sum70[:, :].rearrange("p (c f) -> p c f", c=N_CHUNKS),
                axis=mybir.AxisListType.X, op=mybir.AluOpType.min)

            # ---- transpose [128, 4] -> [4, 128] on PE, DMA psum -> dram
            psT = psbg.tile([4, 128], F32, tag="big")
            nc.tensor.transpose(psT[:, :], loss[:, :], cIdf)
            lossT = ppool.tile([4, 128], F32, tag="lossT")
            nc.vector.tensor_copy(lossT[:, :], psT[:, :])
            nc.sync.dma_start(
                out_d[:].rearrange("(c p) -> c p", p=128), lossT[:, :])

    nc.compile()
    return nc


_NC = None


def _get_nc():
    global _NC
    if _NC is None:
        _NC = build_nc()
    return _NC


def _pack_inputs(pred_corners, target_corners):
    import ml_dtypes

    sr, late = _build_constants()
    pred = np.ascontiguousarray(pred_corners, dtype=np.float32)
    targ = np.ascontiguousarray(target_corners, dtype=np.float32)
    # X rows: 0:24 pred (i*3+c), 24:48 -targ (j*3+c); cols: samples
    xs = np.empty((B_TOTAL, 48), dtype=np.float32)
    xs[:, 0:24] = pred.reshape(B_TOTAL, 24)
    xs[:, 24:48] = -targ.reshape(B_TOTAL, 24)
    xs_bf = xs.astype(ml_dtypes.bfloat16)

    ident = np.eye(128, dtype=np.float32)
    in_maps = []
    for k in range(N_CORES):
        early = np.zeros((128, 512 + _SR_W), dtype=ml_dtypes.bfloat16)
        early[0:48, 0:512] = xs_bf[k * B_CORE:(k + 1) * B_CORE].T
        early[:, 512:] = sr
        in_maps.append({"early": early, "late": late, "ident": ident})
    return in_maps


def kernel(pred_corners: np.ndarray, target_corners: np.ndarray) -> np.ndarray:
    from concourse.bass_utils import run_bass_kernel_spmd

    nc = _get_nc()
    in_maps = _pack_inputs(pred_corners, target_corners)
    res = run_bass_kernel_spmd(nc, in_maps, core_ids=list(range(N_CORES)))
    return np.concatenate([res.results[k]["out"] for k in range(N_CORES)])


# revision 24
# speedup vs baseline: 1.1153x; 1.0004x over previous
"""Trainium2 Bass kernel for CornerBoundingBoxEMDLoss.

For each sample: 8x8 pairwise corner distances, then exact min-cost perfect
matching via meet-in-the-middle (pairs -> quads -> complement pairing), same
math as the reference's 40320-permutation brute force, ~50x less arithmetic.

v3 layout: coord-major [feature, sample], so the distance computation is pure
PE GEMMs with one-hot selection matrices. The pre-matching stage is split
into two 256-sample halves (separate PSUM banks per half) so PE/ACT/DVE
pipeline instead of ping-ponging:

  X [48, 512]      = [pred(i,c); -targ(j,c)] x samples   (bf16, host-packed)
  per half h (256 samples):
    diff = S.T @ X[:,h]      -> psA/psB [96, 256]        (PE)
    sq   = Square(diff)      -> bf16 sbuf                (ACT)
    d2   = R.T @ sq (accum)  -> psC [64, 256]            (PE)
    dist = Sqrt(d2)          -> bf16 sbuf                (ACT)
    L1   = ordering GEMMs    -> psL1 [112, 2*256]        (PE)
    cpy  = Copy(psL1)        -> bf16 sbuf                (ACT)
    m    = TTmin(cpy o0,o1)  -> [112, 256] bf16          (DVE, 2x mode)
  per chunk c (128 samples): L2 GEMMs [128, 840] psum -> DVE min-over-6 ->
    gpsimd A+B add -> DVE min-over-70 -> loss[:, c] bf16
  out: PE-transpose loss [128,4] -> [4,128], ACT copy, one contiguous DMA.

All GEMMs bf16 (1 cyc/row vs fp32's 4 + LOW/HIGH split). Rel err ~5e-3 vs
tolerance 2e-2. Data-parallel across 8 cores, 512 samples each.
"""

import itertools

import numpy as np

import concourse.bacc as bacc
import concourse.mybir as mybir
import concourse.tile as tile

N_CORES = 8
B_TOTAL = 4096
B_CORE = B_TOTAL // N_CORES          # 512
N_CHUNKS = 4
CHUNK = B_CORE // N_CHUNKS           # 128
HALF = B_CORE // 2                   # 256

F32 = mybir.dt.float32
BF16 = mybir.dt.bfloat16

# early tensor ([128, 832] bf16): X in rows 0:48 cols 0:512, S0|S1 in rows
# 0:48 cols 512:704, R0|R1 in rows 0:96 cols 704:832.
# late tensor ([112, 1064] bf16): l1o0|l1o1|l2.
_SR_W = 320
_L1A = 0           # l1o0: rows 0:64, 112 cols
_L1B = 112         # l1o1: rows 0:64, 112 cols
_L2 = 224          # l2: rows 0:112, 840 cols
_LATE_W = 1064


def _build_constants():
    """One-hot selection matrices, packed into a [128, _W-512] bf16 block."""
    import ml_dtypes

    # S0/S1: diff[(i,j,c), b] = X[(i,c), b] + X[24+(j,c), b]  (targ pre-negated)
    s0 = np.zeros((48, 96), dtype=np.float32)
    s1 = np.zeros((48, 96), dtype=np.float32)
    for i in range(4):
        for j in range(8):
            for c in range(3):
                m = i * 24 + j * 3 + c
                s0[i * 3 + c, m] = 1
                s0[24 + j * 3 + c, m] = 1
                s1[(i + 4) * 3 + c, m] = 1
                s1[24 + j * 3 + c, m] = 1

    # R0/R1: d2[(i,j), b] = sum_c sq[(i,j,c), b]; R1 accumulates the i>=4
    # half into output partitions 32..63 of the same psum bank.
    r0 = np.zeros((96, 64), dtype=np.float32)
    r1 = np.zeros((96, 64), dtype=np.float32)
    for i in range(4):
        for j in range(8):
            for c in range(3):
                r0[i * 24 + j * 3 + c, i * 8 + j] = 1
                r1[i * 24 + j * 3 + c, 32 + i * 8 + j] = 1

    # L1: pred-pair x target-pair sums, both orderings (q = pred pair block)
    pairs = list(itertools.combinations(range(8), 2))            # 28
    pair_idx = {p: i for i, p in enumerate(pairs)}
    pred_pairs = [(0, 1), (2, 3), (4, 5), (6, 7)]
    l1o0 = np.zeros((64, 112), dtype=np.float32)
    l1o1 = np.zeros((64, 112), dtype=np.float32)
    for q, (i0, i1) in enumerate(pred_pairs):
        for p, (a, b) in enumerate(pairs):
            col = q * 28 + p
            l1o0[i0 * 8 + a, col] = 1; l1o0[i1 * 8 + b, col] = 1
            l1o1[i0 * 8 + b, col] = 1; l1o1[i1 * 8 + a, col] = 1

    # L2: quad-split sums. cols 0:420 = A side (pred pairs 0,1 onto 4-subset
    # T), cols 420:840 = B side (pred pairs 2,3 onto complement of T).
    subs4 = list(itertools.combinations(range(8), 4))            # 70
    l2 = np.zeros((112, 840), dtype=np.float32)
    for t, T in enumerate(subs4):
        for s, S in enumerate(itertools.combinations(T, 2)):
            rest = tuple(sorted(set(T) - set(S)))
            l2[0 * 28 + pair_idx[S], t * 6 + s] = 1
            l2[1 * 28 + pair_idx[rest], t * 6 + s] = 1
        TB = tuple(sorted(set(range(8)) - set(T)))
        for s, S in enumerate(itertools.combinations(TB, 2)):
            rest = tuple(sorted(set(TB) - set(S)))
            l2[2 * 28 + pair_idx[S], 420 + t * 6 + s] = 1
            l2[3 * 28 + pair_idx[rest], 420 + t * 6 + s] = 1

    sr = np.zeros((128, _SR_W), dtype=ml_dtypes.bfloat16)
    sr[0:48, 0:96] = s0; sr[0:48, 96:192] = s1
    sr[0:96, 192:256] = r0; sr[0:96, 256:320] = r1
    late = np.zeros((112, _LATE_W), dtype=ml_dtypes.bfloat16)
    late[0:64, _L1A:_L1A + 112] = l1o0
    late[0:64, _L1B:_L1B + 112] = l1o1
    late[0:112, _L2:_L2 + 840] = l2
    return sr, late


def build_nc():
    import os
    use_gps = os.environ.get("V_GPS", "1") == "1"

    nc = bacc.Bacc("TRN2", target_bir_lowering=False, debug=False)

    early_d = nc.dram_tensor("early", [128, 512 + _SR_W], BF16,
                             kind="ExternalInput")
    late_d = nc.dram_tensor("late", [112, _LATE_W], BF16, kind="ExternalInput")
    id_d = nc.dram_tensor("ident", [128, 128], F32, kind="ExternalInput")
    out_d = nc.dram_tensor("out", [B_CORE], F32, kind="ExternalOutput")

    with tile.TileContext(nc) as tc:
        with (
            tc.tile_pool(name="consts", bufs=1) as cpool,
            tc.tile_pool(name="persist", bufs=1) as ppool,
            tc.tile_pool(name="work", bufs=2) as wpool,
            # 4 one-bank slots: psA/psB per half -> psC per half -> psL1 per
            # half rotate through. 2 two-bank slots: L2 chunks + transpose.
            tc.tile_pool(name="ps_sm", bufs=4, space="PSUM") as pssm,
            tc.tile_pool(name="ps_big", bufs=2, space="PSUM") as psbg,
        ):
            # dummy sqrt first: forces the single act-table load (the sqrt
            # table also covers square+copy) during the input-DMA wait.
            dummy = cpool.tile([128, 2], F32, tag="dummy")
            nc.gpsimd.memset(dummy[:, 0:1], 1.0)
            nc.scalar.activation(dummy[:, 1:2], dummy[:, 0:1],
                                 mybir.ActivationFunctionType.Sqrt)

            # PE warm-up: the HAM clock gate keeps the PE at 1.2 GHz until
            # it has been busy ~3.4us, and the PE would otherwise idle until
            # the input DMA lands (~10us). Chew through garbage matmuls
            # during the wait so the real GEMM chain runs at 2.4 GHz.
            n_warm = int(os.environ.get("V_WARM", "30"))
            if n_warm:
                scratch = cpool.tile([128, 128], BF16, tag="scratch")
                nc.gpsimd.memset(scratch[:, :], 1.0)
                psD = psbg.tile([128, 1024], F32, tag="big", name="psD")
                for _ in range(n_warm):
                    nc.tensor.matmul(psD[:, 0:128], scratch[:, :],
                                     scratch[:, :], start=True, stop=True)

            early = cpool.tile([128, 512 + _SR_W], BF16, tag="early")
            late = cpool.tile([112, _LATE_W], BF16, tag="late")
            identt = cpool.tile([128, 128], F32, tag="identt")
            nc.sync.dma_start(early[:, :], early_d[:, :])
            nc.sync.dma_start(late[:, :], late_d[:, :])
            nc.sync.dma_start(identt[:, :], id_d[:, :])
            cIdf = identt[:, :]

            cX = early[0:48, 0:512]
            cS = [early[0:48, 512:608], early[0:48, 608:704]]
            cR = [early[0:96, 704:768], early[0:96, 768:832]]
            cL1 = [late[0:64, _L1A:_L1A + 112],
                   late[0:64, _L1B:_L1B + 112]]
            cL2 = late[0:112, _L2:_L2 + 840]

            H = [slice(0, HALF), slice(HALF, 2 * HALF)]

            # ---- per-half chain, h0 ops strictly first so half 0 races
            # down to the L2 stage while half 1 fills the engines behind it
            psA = [pssm.tile([96, 512], F32, tag="bank", name=f"psA{h}") for h in range(2)]
            psB = [pssm.tile([96, 512], F32, tag="bank", name=f"psB{h}") for h in range(2)]
            psC = [pssm.tile([64, 512], F32, tag="bank", name=f"psC{h}") for h in range(2)]
            psL = [pssm.tile([112, 512], F32, tag="bank", name=f"psL{h}") for h in range(2)]
            sq0 = [wpool.tile([96, HALF], BF16, tag=f"sq0{h}", name=f"sq0{h}") for h in range(2)]
            sq1 = [wpool.tile([96, HALF], BF16, tag=f"sq1{h}", name=f"sq1{h}") for h in range(2)]
            dist = [wpool.tile([64, HALF], BF16, tag=f"dist{h}", name=f"dist{h}") for h in range(2)]
            m_t = ppool.tile([112, 512], BF16, tag="m")

            for h in range(2):
                nc.tensor.matmul(psA[h][:, 0:HALF], cS[0], cX[:, H[h]],
                                 start=True, stop=True)
            for h in range(2):
                nc.tensor.matmul(psB[h][:, 0:HALF], cS[1], cX[:, H[h]],
                                 start=True, stop=True)
            for h in range(2):
                nc.scalar.activation(sq0[h][:, :], psA[h][:, 0:HALF],
                                     mybir.ActivationFunctionType.Square)
            for h in range(2):
                nc.scalar.activation(sq1[h][:, :], psB[h][:, 0:HALF],
                                     mybir.ActivationFunctionType.Square)
            for h in range(2):
                nc.tensor.matmul(psC[h][:, 0:HALF], cR[0], sq0[h][:, :],
                                 start=True, stop=False)
            for h in range(2):
                nc.tensor.matmul(psC[h][:, 0:HALF], cR[1], sq1[h][:, :],
                                 start=False, stop=True)
            for h in range(2):
                nc.scalar.activation(dist[h][:, :], psC[h][:, 0:HALF],
                                     mybir.ActivationFunctionType.Sqrt)
            for h in range(2):
                nc.tensor.matmul(psL[h][:, 0:HALF], cL1[0], dist[h][:, :],
                                 start=True, stop=True)
            for h in range(2):
                nc.tensor.matmul(psL[h][:, HALF:2 * HALF], cL1[1], dist[h][:, :],
                                 start=True, stop=True)
            for h in range(2):
                v1 = psL[h][:, :].rearrange("p (o b) -> p b o", o=2)
                nc.vector.tensor_reduce(m_t[:, H[h]], v1,
                                        axis=mybir.AxisListType.X,
                                        op=mybir.AluOpType.min)

            # ---- L2 + L3 per chunk of 128 samples ----
            loss = ppool.tile([128, N_CHUNKS], F32, tag="loss")
            sum70 = ppool.tile([128, N_CHUNKS * 70], BF16, tag="sum70")
            for c in range(N_CHUNKS):
                sl = slice(c * CHUNK, (c + 1) * CHUNK)
                ps2 = psbg.tile([128, 1024], F32, tag="big")
                nc.tensor.matmul(ps2[:, 0:420], m_t[:, sl], cL2[:, 0:420],
                                 start=True, stop=True)
                nc.tensor.matmul(ps2[:, 512:932], m_t[:, sl], cL2[:, 420:840],
                                 start=True, stop=True)

                minab = wpool.tile([128, 140], BF16, tag="minab",
                                   name=f"minab{c}")
                v = (ps2[:, :].rearrange("p (h x) -> p h x", h=2)
                     [:, :, 0:420].rearrange("p h (t s) -> p h t s", s=6))
                nc.vector.tensor_reduce(minab[:, :], v,
                                        axis=mybir.AxisListType.X,
                                        op=mybir.AluOpType.min)

                eng = nc.gpsimd if use_gps else nc.vector
                eng.tensor_tensor(sum70[:, c * 70:(c + 1) * 70],
                                  minab[:, 0:70], minab[:, 70:140],
                                  op=mybir.AluOpType.add)

            # single min-over-70 for all four chunks at once
            nc.vector.tensor_reduce(
                loss[:, :], sum70[:, :].rearrange("p (c f) -> p c f", c=N_CHUNKS),
                axis=mybir.AxisListType.X, op=mybir.AluOpType.min)

            # ---- transpose [128, 4] -> [4, 128] on PE, DMA psum -> dram
            psT = psbg.tile([4, 128], F32, tag="big")
            nc.tensor.transpose(psT[:, :], loss[:, :], cIdf)
            lossT = ppool.tile([4, 128], F32, tag="lossT")
            nc.vector.tensor_copy(lossT[:, :], psT[:, :])
            nc.sync.dma_start(
                out_d[:].rearrange("(c p) -> c p", p=128), lossT[:, :])

    nc.compile()
    return nc


_NC = None


def _get_nc():
    global _NC
    if _NC is None:
        _NC = build_nc()
    return _NC


def _pack_inputs(pred_corners, target_corners):
    import ml_dtypes

    sr, late = _build_constants()
    pred = np.ascontiguousarray(pred_corners, dtype=np.float32)
    targ = np.ascontiguousarray(target_corners, dtype=np.float32)
    # X rows: 0:24 pred (i*3+c), 24:48 -targ (j*3+c); cols: samples
    xs = np.empty((B_TOTAL, 48), dtype=np.float32)
    xs[:, 0:24] = pred.reshape(B_TOTAL, 24)
    xs[:, 24:48] = -targ.reshape(B_TOTAL, 24)
    xs_bf = xs.astype(ml_dtypes.bfloat16)

    ident = np.eye(128, dtype=np.float32)
    in_maps = []
    for k in range(N_CORES):
        early = np.zeros((128, 512 + _SR_W), dtype=ml_dtypes.bfloat16)
        early[0:48, 0:512] = xs_bf[k * B_CORE:(k + 1) * B_CORE].T
        early[:, 512:] = sr
        in_maps.append({"early": early, "late": late, "ident": ident})
    return in_maps


def kernel(pred_corners: np.ndarray, target_corners: np.ndarray) -> np.ndarray:
    from concourse.bass_utils import run_bass_kernel_spmd

    nc = _get_nc()
    in_maps = _pack_inputs(pred_corners, target_corners)
    res = run_bass_kernel_spmd(nc, in_maps, core_ids=list(range(N_CORES)))
    return np.concatenate([res.results[k]["out"] for k in range(N_CORES)])


# revision 25
# speedup vs baseline: 1.1432x; 1.0250x over previous
"""Trainium2 Bass kernel for CornerBoundingBoxEMDLoss.

For each sample: 8x8 pairwise corner distances, then exact min-cost perfect
matching via meet-in-the-middle (pairs -> quads -> complement pairing), same
math as the reference's 40320-permutation brute force, ~50x less arithmetic.

v3 layout: coord-major [feature, sample], so the distance computation is pure
PE GEMMs with one-hot selection matrices. The pre-matching stage is split
into two 256-sample halves (separate PSUM banks per half) so PE/ACT/DVE
pipeline instead of ping-ponging:

  X [48, 512]      = [pred(i,c); -targ(j,c)] x samples   (bf16, host-packed)
  per half h (256 samples):
    diff = S.T @ X[:,h]      -> psA/psB [96, 256]        (PE)
    sq   = Square(diff)      -> bf16 sbuf                (ACT)
    d2   = R.T @ sq (accum)  -> psC [64, 256]            (PE)
    dist = Sqrt(d2)          -> bf16 sbuf                (ACT)
    L1   = ordering GEMMs    -> psL1 [112, 2*256]        (PE)
    cpy  = Copy(psL1)        -> bf16 sbuf                (ACT)
    m    = TTmin(cpy o0,o1)  -> [112, 256] bf16          (DVE, 2x mode)
  per chunk c (128 samples): L2 GEMMs [128, 840] psum -> DVE min-over-6 ->
    gpsimd A+B add -> DVE min-over-70 -> loss[:, c] bf16
  out: PE-transpose loss [128,4] -> [4,128], ACT copy, one contiguous DMA.

All GEMMs bf16 (1 cyc/row vs fp32's 4 + LOW/HIGH split). Rel err ~5e-3 vs
tolerance 2e-2. Data-parallel across 8 cores, 512 samples each.
"""

import itertools

import numpy as np

import concourse.bacc as bacc
import concourse.mybir as mybir
import concourse.tile as tile

N_CORES = 8
B_TOTAL = 4096
B_CORE = B_TOTAL // N_CORES          # 512
N_CHUNKS = 4
CHUNK = B_CORE // N_CHUNKS           # 128
HALF = B_CORE // 2                   # 256

F32 = mybir.dt.float32
BF16 = mybir.dt.bfloat16

# early tensor ([128, 832] bf16): X in rows 0:48 cols 0:512, S0|S1 in rows
# 0:48 cols 512:704, R0|R1 in rows 0:96 cols 704:832.
# late tensor ([112, 1064] bf16): l1o0|l1o1|l2.
_SR_W = 320
_L1A = 0           # l1o0: rows 0:64, 112 cols
_L1B = 112         # l1o1: rows 0:64, 112 cols
_L2 = 224          # l2: rows 0:112, 840 cols
_LATE_W = 1064


def _build_constants():
    """One-hot selection matrices, packed into a [128, _W-512] bf16 block."""
    import ml_dtypes

    # S0/S1: diff[(i,j,c), b] = X[(i,c), b] + X[24+(j,c), b]  (targ pre-negated)
    s0 = np.zeros((48, 96), dtype=np.float32)
    s1 = np.zeros((48, 96), dtype=np.float32)
    for i in range(4):
        for j in range(8):
            for c in range(3):
                m = i * 24 + j * 3 + c
                s0[i * 3 + c, m] = 1
                s0[24 + j * 3 + c, m] = 1
                s1[(i + 4) * 3 + c, m] = 1
                s1[24 + j * 3 + c, m] = 1

    # R0/R1: d2[(i,j), b] = sum_c sq[(i,j,c), b]; R1 accumulates the i>=4
    # half into output partitions 32..63 of the same psum bank.
    r0 = np.zeros((96, 64), dtype=np.float32)
    r1 = np.zeros((96, 64), dtype=np.float32)
    for i in range(4):
        for j in range(8):
            for c in range(3):
                r0[i * 24 + j * 3 + c, i * 8 + j] = 1
                r1[i * 24 + j * 3 + c, 32 + i * 8 + j] = 1

    # L1: pred-pair x target-pair sums, both orderings (q = pred pair block)
    pairs = list(itertools.combinations(range(8), 2))            # 28
    pair_idx = {p: i for i, p in enumerate(pairs)}
    pred_pairs = [(0, 1), (2, 3), (4, 5), (6, 7)]
    l1o0 = np.zeros((64, 112), dtype=np.float32)
    l1o1 = np.zeros((64, 112), dtype=np.float32)
    for q, (i0, i1) in enumerate(pred_pairs):
        for p, (a, b) in enumerate(pairs):
            col = q * 28 + p
            l1o0[i0 * 8 + a, col] = 1; l1o0[i1 * 8 + b, col] = 1
            l1o1[i0 * 8 + b, col] = 1; l1o1[i1 * 8 + a, col] = 1

    # L2: quad-split sums. cols 0:420 = A side (pred pairs 0,1 onto 4-subset
    # T), cols 420:840 = B side (pred pairs 2,3 onto complement of T).
    subs4 = list(itertools.combinations(range(8), 4))            # 70
    l2 = np.zeros((112, 840), dtype=np.float32)
    for t, T in enumerate(subs4):
        for s, S in enumerate(itertools.combinations(T, 2)):
            rest = tuple(sorted(set(T) - set(S)))
            l2[0 * 28 + pair_idx[S], t * 6 + s] = 1
            l2[1 * 28 + pair_idx[rest], t * 6 + s] = 1
        TB = tuple(sorted(set(range(8)) - set(T)))
        for s, S in enumerate(itertools.combinations(TB, 2)):
            rest = tuple(sorted(set(TB) - set(S)))
            l2[2 * 28 + pair_idx[S], 420 + t * 6 + s] = 1
            l2[3 * 28 + pair_idx[rest], 420 + t * 6 + s] = 1

    sr = np.zeros((128, _SR_W), dtype=ml_dtypes.bfloat16)
    sr[0:48, 0:96] = s0; sr[0:48, 96:192] = s1
    sr[0:96, 192:256] = r0; sr[0:96, 256:320] = r1
    late = np.zeros((112, _LATE_W), dtype=ml_dtypes.bfloat16)
    late[0:64, _L1A:_L1A + 112] = l1o0
    late[0:64, _L1B:_L1B + 112] = l1o1
    late[0:112, _L2:_L2 + 840] = l2
    return sr, late


def build_nc():
    import os
    use_gps = os.environ.get("V_GPS", "1") == "1"

    nc = bacc.Bacc("TRN2", target_bir_lowering=False, debug=False)

    early_d = nc.dram_tensor("early", [128, 512 + _SR_W], BF16,
                             kind="ExternalInput")
    late_d = nc.dram_tensor("late", [112, _LATE_W], BF16, kind="ExternalInput")
    out_d = nc.dram_tensor("out", [128, N_CHUNKS], F32, kind="ExternalOutput")

    with tile.TileContext(nc) as tc:
        with (
            tc.tile_pool(name="consts", bufs=1) as cpool,
            tc.tile_pool(name="persist", bufs=1) as ppool,
            tc.tile_pool(name="work", bufs=2) as wpool,
            # 4 one-bank slots: psA/psB per half -> psC per half -> psL1 per
            # half rotate through. 2 two-bank slots: L2 chunks + transpose.
            tc.tile_pool(name="ps_sm", bufs=4, space="PSUM") as pssm,
            tc.tile_pool(name="ps_big", bufs=2, space="PSUM") as psbg,
        ):
            # dummy sqrt first: forces the single act-table load (the sqrt
            # table also covers square+copy) during the input-DMA wait.
            dummy = cpool.tile([128, 2], F32, tag="dummy")
            nc.gpsimd.memset(dummy[:, 0:1], 1.0)
            nc.scalar.activation(dummy[:, 1:2], dummy[:, 0:1],
                                 mybir.ActivationFunctionType.Sqrt)

            # PE warm-up: the HAM clock gate keeps the PE at 1.2 GHz until
            # it has been busy ~3.4us, and the PE would otherwise idle until
            # the input DMA lands (~10us). Chew through garbage matmuls
            # during the wait so the real GEMM chain runs at 2.4 GHz.
            n_warm = int(os.environ.get("V_WARM", "26"))
            if n_warm:
                scratch = cpool.tile([128, 128], BF16, tag="scratch")
                nc.gpsimd.memset(scratch[:, :], 1.0)
                psD = psbg.tile([128, 1024], F32, tag="big", name="psD")
                for _ in range(n_warm):
                    nc.tensor.matmul(psD[:, 0:128], scratch[:, :],
                                     scratch[:, :], start=True, stop=True)

            early = cpool.tile([128, 512 + _SR_W], BF16, tag="early")
            late = cpool.tile([112, _LATE_W], BF16, tag="late")
            nc.sync.dma_start(early[:, :], early_d[:, :])
            nc.sync.dma_start(late[:, :], late_d[:, :])

            cX = early[0:48, 0:512]
            cS = [early[0:48, 512:608], early[0:48, 608:704]]
            cR = [early[0:96, 704:768], early[0:96, 768:832]]
            cL1 = [late[0:64, _L1A:_L1A + 112],
                   late[0:64, _L1B:_L1B + 112]]
            cL2 = late[0:112, _L2:_L2 + 840]

            H = [slice(0, HALF), slice(HALF, 2 * HALF)]

            # ---- per-half chain, h0 ops strictly first so half 0 races
            # down to the L2 stage while half 1 fills the engines behind it
            psA = [pssm.tile([96, 512], F32, tag="bank", name=f"psA{h}") for h in range(2)]
            psB = [pssm.tile([96, 512], F32, tag="bank", name=f"psB{h}") for h in range(2)]
            psC = [pssm.tile([64, 512], F32, tag="bank", name=f"psC{h}") for h in range(2)]
            psL = [pssm.tile([112, 512], F32, tag="bank", name=f"psL{h}") for h in range(2)]
            sq0 = [wpool.tile([96, HALF], BF16, tag=f"sq0{h}", name=f"sq0{h}") for h in range(2)]
            sq1 = [wpool.tile([96, HALF], BF16, tag=f"sq1{h}", name=f"sq1{h}") for h in range(2)]
            dist = [wpool.tile([64, HALF], BF16, tag=f"dist{h}", name=f"dist{h}") for h in range(2)]
            m_t = ppool.tile([112, 512], BF16, tag="m")

            for h in range(2):
                nc.tensor.matmul(psA[h][:, 0:HALF], cS[0], cX[:, H[h]],
                                 start=True, stop=True)
            for h in range(2):
                nc.tensor.matmul(psB[h][:, 0:HALF], cS[1], cX[:, H[h]],
                                 start=True, stop=True)
            for h in range(2):
                nc.scalar.activation(sq0[h][:, :], psA[h][:, 0:HALF],
                                     mybir.ActivationFunctionType.Square)
                nc.scalar.activation(sq1[h][:, :], psB[h][:, 0:HALF],
                                     mybir.ActivationFunctionType.Square)
            for h in range(2):
                nc.tensor.matmul(psC[h][:, 0:HALF], cR[0], sq0[h][:, :],
                                 start=True, stop=False)
            for h in range(2):
                nc.tensor.matmul(psC[h][:, 0:HALF], cR[1], sq1[h][:, :],
                                 start=False, stop=True)
            for h in range(2):
                nc.scalar.activation(dist[h][:, :], psC[h][:, 0:HALF],
                                     mybir.ActivationFunctionType.Sqrt)
            for h in range(2):
                nc.tensor.matmul(psL[h][:, 0:HALF], cL1[0], dist[h][:, :],
                                 start=True, stop=True)
            for h in range(2):
                nc.tensor.matmul(psL[h][:, HALF:2 * HALF], cL1[1], dist[h][:, :],
                                 start=True, stop=True)
            for h in range(2):
                v1 = psL[h][:, :].rearrange("p (o b) -> p b o", o=2)
                nc.vector.tensor_reduce(m_t[:, H[h]], v1,
                                        axis=mybir.AxisListType.X,
                                        op=mybir.AluOpType.min)

            # ---- L2 + L3 per chunk of 128 samples ----
            loss = ppool.tile([128, N_CHUNKS], F32, tag="loss")
            sum70 = ppool.tile([128, N_CHUNKS * 70], BF16, tag="sum70")
            for c in range(N_CHUNKS):
                sl = slice(c * CHUNK, (c + 1) * CHUNK)
                ps2 = psbg.tile([128, 1024], F32, tag="big")
                nc.tensor.matmul(ps2[:, 0:420], m_t[:, sl], cL2[:, 0:420],
                                 start=True, stop=True)
                nc.tensor.matmul(ps2[:, 512:932], m_t[:, sl], cL2[:, 420:840],
                                 start=True, stop=True)

                minab = wpool.tile([128, 140], BF16, tag="minab",
                                   name=f"minab{c}")
                v = (ps2[:, :].rearrange("p (h x) -> p h x", h=2)
                     [:, :, 0:420].rearrange("p h (t s) -> p h t s", s=6))
                nc.vector.tensor_reduce(minab[:, :], v,
                                        axis=mybir.AxisListType.X,
                                        op=mybir.AluOpType.min)

                eng = nc.gpsimd if (use_gps and c < N_CHUNKS - 1) else nc.vector
                eng.tensor_tensor(sum70[:, c * 70:(c + 1) * 70],
                                  minab[:, 0:70], minab[:, 70:140],
                                  op=mybir.AluOpType.add)

            # single min-over-70 for all four chunks at once
            nc.vector.tensor_reduce(
                loss[:, :], sum70[:, :].rearrange("p (c f) -> p c f", c=N_CHUNKS),
                axis=mybir.AxisListType.X, op=mybir.AluOpType.min)

            # ---- loss [128, 4] DMAs out as 16B/partition (contiguous,
            # fast); the host un-interleaves to sample order.
            nc.sync.dma_start(out_d[:, :], loss[:, :])

    nc.compile()
    return nc


_NC = None


def _get_nc():
    global _NC
    if _NC is None:
        _NC = build_nc()
    return _NC


def _pack_inputs(pred_corners, target_corners):
    import ml_dtypes

    sr, late = _build_constants()
    pred = np.ascontiguousarray(pred_corners, dtype=np.float32)
    targ = np.ascontiguousarray(target_corners, dtype=np.float32)
    # X rows: 0:24 pred (i*3+c), 24:48 -targ (j*3+c); cols: samples
    xs = np.empty((B_TOTAL, 48), dtype=np.float32)
    xs[:, 0:24] = pred.reshape(B_TOTAL, 24)
    xs[:, 24:48] = -targ.reshape(B_TOTAL, 24)
    xs_bf = xs.astype(ml_dtypes.bfloat16)

    in_maps = []
    for k in range(N_CORES):
        early = np.zeros((128, 512 + _SR_W), dtype=ml_dtypes.bfloat16)
        early[0:48, 0:512] = xs_bf[k * B_CORE:(k + 1) * B_CORE].T
        early[:, 512:] = sr
        in_maps.append({"early": early, "late": late})
    return in_maps


def kernel(pred_corners: np.ndarray, target_corners: np.ndarray) -> np.ndarray:
    from concourse.bass_utils import run_bass_kernel_spmd

    nc = _get_nc()
    in_maps = _pack_inputs(pred_corners, target_corners)
    res = run_bass_kernel_spmd(nc, in_maps, core_ids=list(range(N_CORES)))
    # out[p, c] holds the loss of sample c*128+p on each core
    return np.concatenate(
        [res.results[k]["out"].T.reshape(B_CORE) for k in range(N_CORES)])
